# revision 12
# baseline (speedup 1.0000x reference)
"""3-layer GAT on 8 TRN2 NeuronCores — degree-sorted identity packing.

Strategy (graph/data parallel):
- Nodes are relabeled by in-degree (desc) and dealt into 160 blocks of 128
  (8 cores x 20 blocks): rank r -> core (r%1024)//128, block r//1024,
  slot r%128. Within a block all nodes have near-equal degree, so per-block
  edge slots [slot p, q] with q < Q_b = max degree in block waste ~12%.
- Edge slot [p, q] of block b holds the q-th incoming edge of the node at
  slot p. Aggregation = PSUM-accumulated identity matmuls over q (segment
  sum with zero bookkeeping); al_d is a local per-block column broadcast;
  only SRC rows are gathered (halves DMA vs dst+src gathering).
- Per layer: local feature matmul produces h|al_s|al_d rows; rows
  AllGathered into a global [20480, row] table; per-edge src rows fetched
  with dma_gather (4 SWDGE queues). Pad slots point at a table row whose
  al_s is -1e4, so exp(lrelu(...)) == 0 kills their contribution.
- Softmax max-subtraction is shift-invariant and dropped (|e| <= ~10).
"""
import sys
sys.path.insert(0, "/opt/trn_rl_repo")
import numpy as np
import ml_dtypes

import concourse.bass as bass
import concourse.tile as tile
from concourse import bacc, mybir
from concourse.bass_utils import run_bass_kernel_spmd
from concourse.masks import make_identity

BF16 = ml_dtypes.bfloat16
N = 20000
E = 320000
FIN = 1024
H = 4
C = 64
NCLS = 4
NEG = 0.2
NCORE = 8
NPAD = 2560           # per-core node rows (20 blocks of 128)
NBLK = NPAD // 128
SLAB = NCORE * 128    # nodes per block-row across all cores
NG = NCORE * NPAD     # global padded table rows
ROW12 = 384           # bf16 cols: h 0:256, al_s 256:260, al_d 260:264, pad
ROW3 = 128            # bf16 cols: h 0:64, al_s 64, al_d 65, pad

_cache = {}


def _prep_edges(src, dst):
    """Degree-sorted relabeling + identity-packed per-block gather indices.

    Returns (Qb, per_core_ix, row_of_node, node_of_row, colmask) where
    Qb[b] = slot depth of block b (shared across cores), per_core_ix[c] =
    [128, 8*sum(Qb)] int16 wrapped gather indices, row_of_node[n] = global
    padded table row, node_of_row[c] = original node id per local row (-1
    pad), colmask[c] = [128, NBLK] f32 (0 real, -1e4 pad)."""
    deg = np.bincount(dst, minlength=N)
    order = np.argsort(-deg, kind="stable")      # rank -> node
    rank = np.empty(N, np.int64)
    rank[order] = np.arange(N)
    core_of = (rank % SLAB) // 128
    blk_of = rank // SLAB
    slot_of = rank % 128
    # row 2559 of every core is the poison pad row for dma_gather padding;
    # relocate any real node assigned there to a free row of another core
    # (same degree slab, so Qb bounds still hold)
    local = blk_of * 128 + slot_of
    used = np.zeros((NCORE, NPAD), bool)
    used[core_of, local] = True
    for n in np.where(local == NPAD - 1)[0]:
        c2 = int(np.argmin(used.sum(1)))
        free = np.where(~used[c2, :NPAD - 1])[0]
        r2 = int(free[-1])              # a block-19 row, degree class safe
        used[core_of[n], NPAD - 1] = False
        used[c2, r2] = True
        core_of[n], blk_of[n], slot_of[n] = c2, r2 // 128, r2 % 128
    # half-split AllGather layout: blocks 0-9 of all cores first (rows
    # c*1280 + local), then blocks 10-19 (10240 + c*1280 + local-1280)
    half = (blk_of >= NBLK // 2).astype(np.int64)
    row_of_node = (half * (NG // 2) + core_of * (NPAD // 2)
                   + (blk_of - half * (NBLK // 2)) * 128 + slot_of)

    degs = deg[order]                            # degree by rank, desc
    Qb = [max(1, int(degs[b * SLAB])) for b in range(NBLK)]
    offs = np.concatenate([[0], np.cumsum([128 * q for q in Qb])])
    tot = int(offs[-1])

    # per-edge q = incoming counter of its dst
    eorder = np.argsort(dst, kind="stable")
    sdst = dst[eorder]
    ssrc = src[eorder]
    first = np.searchsorted(sdst, sdst, side="left")
    q = np.arange(len(sdst)) - first
    c_e = core_of[sdst]
    pos = offs[blk_of[sdst]] + q * 128 + slot_of[sdst]

    node_of_row = np.full((NCORE, NPAD), -1, np.int64)
    node_of_row[core_of, blk_of * 128 + slot_of] = np.arange(N)

    colmask = np.zeros((NCORE, 128, NBLK), np.float32)
    padrow = node_of_row < 0                       # [NCORE, NPAD]
    colmask[padrow.reshape(NCORE, NBLK, 128).transpose(0, 2, 1)] = -1.0e4

    per_core_ix = []
    for c in range(NCORE):
        padrow_g = NG // 2 + c * (NPAD // 2) + NPAD // 2 - 1  # own row 2559
        ix = np.full(tot, padrow_g, np.int64)
        m = c_e == c
        ix[pos[m]] = row_of_node[ssrc[m]]
        # wrap to dma_gather layout [16, tot//16] tiled x8 partitions
        w = ix.reshape(tot // 16, 16).T.astype(np.int16)
        per_core_ix.append(np.tile(w, (8, 1)))
    return Qb, per_core_ix, row_of_node, node_of_row, colmask


def _fold_w(W, a_s, a_d):
    """[F, H*C] weights + per-head a vectors -> [F, H*C + 2H] f32."""
    F = W.shape[0]
    Hh, Cc = a_s.shape
    As = np.zeros((Hh * Cc, Hh), np.float64)
    Ad = np.zeros((Hh * Cc, Hh), np.float64)
    for h in range(Hh):
        As[h * Cc:(h + 1) * Cc, h] = a_s[h]
        Ad[h * Cc:(h + 1) * Cc, h] = a_d[h]
    W64 = W.astype(np.float64)
    return np.concatenate([W64, W64 @ As, W64 @ Ad], axis=1).astype(np.float32)


def _build(Qb):
    dt = mybir.dt
    QMAX = max(Qb)
    offs = [int(v) for v in np.concatenate([[0], np.cumsum([128 * q for q in Qb])])]
    IXC = offs[-1] // 16                 # = 8*sum(Qb)
    nc = bacc.Bacc("TRN2", num_devices=NCORE, debug=False, num_swdge_queues=4)

    xt_in = nc.dram_tensor("xt", [FIN, NPAD], dt.bfloat16, kind="ExternalInput")
    w1e_in = nc.dram_tensor("w1e", [FIN, 264], dt.bfloat16, kind="ExternalInput")
    w2e_in = nc.dram_tensor("w2e", [256, 264], dt.bfloat16, kind="ExternalInput")
    w3e_in = nc.dram_tensor("w3e", [256, 66], dt.bfloat16, kind="ExternalInput")
    wc_in = nc.dram_tensor("wc", [64, 4], dt.bfloat16, kind="ExternalInput")
    b1_in = nc.dram_tensor("b1r", [128, 256], dt.bfloat16, kind="ExternalInput")
    b2_in = nc.dram_tensor("b2r", [128, 256], dt.bfloat16, kind="ExternalInput")
    b3_in = nc.dram_tensor("b3r", [128, 64], dt.bfloat16, kind="ExternalInput")
    bc_in = nc.dram_tensor("bcr", [128, 4], dt.float32, kind="ExternalInput")
    ixs_in = nc.dram_tensor("ixs", [128, IXC], dt.int16, kind="ExternalInput")
    cm_in = nc.dram_tensor("cm", [128, NBLK], dt.float32, kind="ExternalInput")
    out_d = nc.dram_tensor("out", [NPAD, 4], dt.float32, kind="ExternalOutput")

    tabin = [nc.dram_tensor(f"tabin{l}", [NPAD, r], dt.bfloat16, kind="Internal")
             for l, r in ((1, ROW12), (2, ROW12), (3, ROW3))]
    tabg = [nc.dram_tensor(f"tabg{l}", [NG, r], dt.bfloat16, kind="Internal",
                           addr_space="Shared")
            for l, r in ((1, ROW12), (2, ROW12), (3, ROW3))]
    tabg3n = nc.dram_tensor("tabg3n", [NG, 66], dt.bfloat16, kind="Internal",
                            addr_space="Shared")
    tabin3n = nc.dram_tensor("tabin3n", [NPAD, 66], dt.bfloat16, kind="Internal")

    with tile.TileContext(nc) as tc:
        with (
            tc.tile_pool(name="const", bufs=1) as cpool,
            tc.tile_pool(name="work", bufs=2) as wpool,
            tc.tile_pool(name="gbuf", bufs=3) as gpool,
            tc.tile_pool(name="psum", bufs=2, space="PSUM") as ppool,
        ):
            # ---- constants to SBUF
            ident = cpool.tile([128, 128], dt.bfloat16)
            make_identity(nc, ident[:])
            ixs = cpool.tile([128, IXC], dt.int16)
            nc.sync.dma_start(out=ixs[:], in_=ixs_in[:])
            cmask = cpool.tile([128, NBLK], dt.float32)
            nc.sync.dma_start(out=cmask[:], in_=cm_in[:])
            w1e = cpool.tile([128, 8, 264], dt.bfloat16)
            nc.sync.dma_start(out=w1e[:], in_=w1e_in[:].rearrange("(k p) c -> p k c", p=128))
            w2e = cpool.tile([128, 2, 264], dt.bfloat16)
            nc.sync.dma_start(out=w2e[:], in_=w2e_in[:].rearrange("(k p) c -> p k c", p=128))
            w3e = cpool.tile([128, 2, 66], dt.bfloat16)
            nc.sync.dma_start(out=w3e[:], in_=w3e_in[:].rearrange("(k p) c -> p k c", p=128))
            wc = cpool.tile([64, 4], dt.bfloat16)
            nc.sync.dma_start(out=wc[:], in_=wc_in[:])
            b1r = cpool.tile([128, 256], dt.bfloat16)
            nc.sync.dma_start(out=b1r[:], in_=b1_in[:])
            b2r = cpool.tile([128, 256], dt.bfloat16)
            nc.sync.dma_start(out=b2r[:], in_=b2_in[:])
            b3r = cpool.tile([128, 64], dt.bfloat16)
            nc.sync.dma_start(out=b3r[:], in_=b3_in[:])
            bcr = cpool.tile([128, 4], dt.float32)
            nc.sync.dma_start(out=bcr[:], in_=bc_in[:])

            # xT buffers for layers 2/3 outputs
            xt2 = cpool.tile([128, 2, NPAD], dt.bfloat16)
            xt2b = cpool.tile([128, 2, NPAD], dt.bfloat16)
            xt3 = cpool.tile([64, NPAD], dt.bfloat16)
            # local al_d per layer: [128, NBLK, hh]
            ald1 = cpool.tile([128, NBLK, H], dt.float32)
            ald2 = cpool.tile([128, NBLK, H], dt.float32)
            ald3 = cpool.tile([128, NBLK, 1], dt.float32)

            def phase_a_blk(m, xt_sb, kc, wext, cols, tab_in, hcols, hh,
                            aldloc, xtm2box=None):
                """h|al = x @ Wext for one 128-node chunk; write table rows,
                stash local al_d, poison pad-row al_s. xt_sb=None streams
                layer-1 x^T blocks from DRAM (saves 40KB/partition SBUF)."""
                if True:
                    if xt_sb is None:
                        if m % 2 == 0:
                            xtmt = wpool.tile([128, 8, 256], dt.bfloat16,
                                              tag="xtm")
                            xtm2box[0] = xtmt
                            nc.scalar.dma_start(
                                out=xtmt[:],
                                in_=xt_in[:].rearrange("(k p) n -> p k n", p=128)
                                [:, :, m * 128:(m + 2) * 128])
                        xtm = xtm2box[0][:, :, (m % 2) * 128:(m % 2 + 1) * 128]
                    ps = ppool.tile([128, cols], dt.float32, tag="psA")
                    for k in range(kc):
                        if xt_sb is None:
                            lhsT = xtm[:, k, :]
                        else:
                            lhsT = (xt_sb[:, k, m * 128:(m + 1) * 128] if kc > 1
                                    else xt_sb[:, m * 128:(m + 1) * 128])
                        nc.tensor.matmul(ps[:], lhsT, wext[:, k, :] if kc > 1 else wext[:],
                                         start=(k == 0), stop=(k == kc - 1))
                    hrow = wpool.tile([128, hcols + hh], dt.bfloat16, tag="hrow")
                    nc.vector.tensor_copy(hrow[:, 0:hcols], ps[:, 0:hcols])
                    # al_s + pad poison (colmask is 0 for real rows, -1e4 pad)
                    nc.vector.tensor_tensor(
                        out=hrow[:, hcols:hcols + hh],
                        in0=ps[:, hcols:hcols + hh],
                        in1=cmask[:, m:m + 1].to_broadcast([128, hh]),
                        op=mybir.AluOpType.add)
                    nc.vector.tensor_copy(aldloc[:, m, :], ps[:, hcols + hh:hcols + 2 * hh])
                    nc.sync.dma_start(out=tab_in[m * 128:(m + 1) * 128, 0:hcols + hh],
                                      in_=hrow[:])

            def phase_a(xt_sb, kc, wext, cols, tab_in, hcols, hh, aldloc,
                        coll=None):
                box = [None]
                for m in range(NBLK):
                    phase_a_blk(m, xt_sb, kc, wext, cols, tab_in, hcols, hh,
                                aldloc, box)
                    if coll is not None and m == NBLK // 2 - 1:
                        coll(0)
                if coll is not None:
                    coll(1)

            def half_ag(tab_in, tab_out, cols, expand=None):
                HN, HGN = NPAD // 2, NG // 2
                def go(half):
                    nc.gpsimd.collective_compute(
                        "AllGather", mybir.AluOpType.bypass,
                        replica_groups=[list(range(NCORE))],
                        ins=[tab_in[half * HN:(half + 1) * HN, 0:cols]],
                        outs=[tab_out[half * HGN:(half + 1) * HGN, :]])
                    if expand is not None:
                        nc.sync.dma_start(
                            out=expand[half * HGN:(half + 1) * HGN, 0:cols],
                            in_=tab_out[half * HGN:(half + 1) * HGN, :])
                return go

            def classifier_blk(m):
                ps = ppool.tile([128, 4], dt.float32, tag="psD")
                nc.tensor.matmul(ps[:], xt3[:, m * 128:(m + 1) * 128], wc[:],
                                 start=True, stop=True)
                of = wpool.tile([128, 4], dt.float32, tag="of")
                nc.vector.tensor_tensor(out=of[:], in0=ps[:], in1=bcr[:],
                                        op=mybir.AluOpType.add)
                nc.sync.dma_start(out=out_d[m * 128:(m + 1) * 128, :], in_=of[:])

            def edge_phase(l, tab, row, hcols, hh, xt_out, brep, do_relu,
                           aldloc, next_cb=None, next_coll=None):
                """3-stage software pipeline per dst-block:
                gather(b+2) | attention smalls(b+1) | heavy payload ops(b);
                next_cb(b) interleaves the next layer's per-block work."""
                mcols = hcols + hh

                def gath(b):
                    Q = Qb[b]
                    g = gpool.tile([128, QMAX, row], dt.bfloat16, tag="g")
                    nc.gpsimd.dma_gather(
                        out_ap=g[:, 0:Q, :], in_ap=tab[:],
                        idxs_ap=ixs[:, offs[b] // 16:offs[b] // 16 + Q * 8],
                        num_idxs=Q * 128, num_idxs_reg=Q * 128, elem_size=row,
                        single_packet=False, queue_num=b % 4)
                    return g

                def smalls_add(b, g):
                    # e = al_s[src] + al_d[dst]; two exps on ScalarE
                    Q = Qb[b]
                    ew = wpool.tile([128, QMAX * hh], dt.float32, tag="ew")
                    nc.vector.tensor_tensor(
                        out=ew[:, 0:Q * hh].rearrange("p (q h) -> p q h", h=hh),
                        in0=g[:, 0:Q, hcols:hcols + hh],
                        in1=aldloc[:, b, None, :].to_broadcast([128, Q, hh]),
                        op=mybir.AluOpType.add)
                    ew2 = wpool.tile([128, QMAX * hh], dt.float32, tag="ew2")
                    nc.scalar.activation(ew2[:, 0:Q * hh], ew[:, 0:Q * hh],
                                         mybir.ActivationFunctionType.Exp)
                    nc.scalar.activation(ew[:, 0:Q * hh], ew[:, 0:Q * hh],
                                         mybir.ActivationFunctionType.Exp,
                                         scale=NEG)
                    return ew, ew2

                def smalls_max(b, g, ew, ew2):
                    # w = exp(lrelu(e)) = max(exp(e), exp(0.2*e)) -> g w cols
                    Q = Qb[b]
                    nc.vector.tensor_tensor(
                        out=g[:, 0:Q, hcols:hcols + hh],
                        in0=ew[:, 0:Q * hh].rearrange("p (q h) -> p q h", h=hh),
                        in1=ew2[:, 0:Q * hh].rearrange("p (q h) -> p q h", h=hh),
                        op=mybir.AluOpType.max)

                def heavy(b, g):
                    Q = Qb[b]
                    # expanded w -> per-channel weights, two-stage on ScalarE:
                    # 1x broadcast to 8 wide, then unit-stride replicate to 64
                    wr8 = wpool.tile([128, QMAX, hh, 8], dt.bfloat16, tag="wr8")
                    nc.scalar.activation(
                        wr8[:, 0:Q],
                        g[:, 0:Q, hcols:hcols + hh][:, :, :, None]
                        .to_broadcast([128, Q, hh, 8]),
                        mybir.ActivationFunctionType.Copy)
                    wex = wpool.tile([128, QMAX, hcols], dt.bfloat16, tag="wex")
                    nc.vector.tensor_copy(
                        wex[:, 0:Q, :].rearrange("p q (h r e) -> p q h r e", r=8, e=8),
                        wr8[:, 0:Q][:, :, :, None, :]
                        .to_broadcast([128, Q, hh, 8, 8]))
                    nc.vector.tensor_tensor(
                        out=g[:, 0:Q, 0:hcols], in0=g[:, 0:Q, 0:hcols],
                        in1=wex[:, 0:Q, :], op=mybir.AluOpType.mult)
                    ps = ppool.tile([128, mcols], dt.float32, tag="psC")
                    for s in range(Q):
                        nc.tensor.matmul(ps[:], ident[:], g[:, s, 0:mcols],
                                         start=(s == 0), stop=(s == Q - 1))
                    # normalize, bias, relu
                    den = wpool.tile([128, hh], dt.float32, tag="den")
                    nc.vector.tensor_scalar_add(den[:], ps[:, hcols:mcols], 1e-16)
                    nc.vector.reciprocal_approx_fast(den[:], den[:])
                    x2 = wpool.tile([128, hcols], dt.bfloat16, tag="x2")
                    nc.vector.tensor_tensor(
                        out=x2[:].rearrange("p (h c) -> p h c", c=C),
                        in0=ps[:, 0:hcols].rearrange("p (h c) -> p h c", c=C),
                        in1=den[:][:, :, None].to_broadcast([128, hh, C]),
                        op=mybir.AluOpType.mult)
                    nc.vector.tensor_tensor(out=x2[:], in0=x2[:], in1=brep[:],
                                            op=mybir.AluOpType.add)
                    if do_relu:
                        nc.vector.tensor_scalar_max(x2[:], x2[:], 0.0)
                    # transpose into xt_out columns (psum->sbuf copy on ScalarE)
                    nhalf = hcols // 128
                    if nhalf == 0:
                        tp = ppool.tile([hcols, 128], dt.bfloat16, tag="tp")
                        nc.tensor.transpose(tp[:], x2[:], ident[:])
                        nc.scalar.activation(
                            xt_out[:, b * 128:(b + 1) * 128], tp[:],
                            mybir.ActivationFunctionType.Copy)
                    else:
                        for hf in range(nhalf):
                            tp = ppool.tile([128, 128], dt.bfloat16, tag="tp")
                            nc.tensor.transpose(tp[:], x2[:, hf * 128:(hf + 1) * 128],
                                                ident[:])
                            nc.scalar.activation(
                                xt_out[:, hf, b * 128:(b + 1) * 128], tp[:],
                                mybir.ActivationFunctionType.Copy)

                # prologue
                gs = {0: gath(0)}
                if NBLK > 1:
                    gs[1] = gath(1)
                ews = {0: smalls_add(0, gs[0])}
                smalls_max(0, gs[0], *ews.pop(0))
                for b in range(NBLK):
                    if b + 2 < NBLK:
                        gs[b + 2] = gath(b + 2)
                    if b + 1 < NBLK:
                        ews[b + 1] = smalls_add(b + 1, gs[b + 1])
                    heavy(b, gs.pop(b))
                    if next_cb is not None:
                        next_cb(b)
                        if next_coll is not None and b == NBLK // 2 - 1:
                            next_coll(0)
                    if b + 1 < NBLK:
                        smalls_max(b + 1, gs[b + 1], *ews.pop(b + 1))
                if next_coll is not None:
                    next_coll(1)

            # ===== layer 1 (first-half AllGather overlaps phase_a tail)
            ag1 = half_ag(tabin[0], tabg[0], ROW12)
            phase_a(None, 8, w1e, 264, tabin[0], 256, H, ald1, coll=ag1)
            ag2 = half_ag(tabin[1], tabg[1], ROW12)
            edge_phase(1, tabg[0], ROW12, 256, H, xt2, b1r, True, ald1,
                       next_cb=lambda b: phase_a_blk(
                           b, xt2, 2, w2e, 264, tabin[1], 256, H, ald2),
                       next_coll=ag2)

            # ===== layer 2
            ag3 = half_ag(tabin3n, tabg3n, 66, expand=tabg[2])
            edge_phase(2, tabg[1], ROW12, 256, H, xt2b, b2r, True, ald2,
                       next_cb=lambda b: phase_a_blk(
                           b, xt2b, 2, w3e, 66, tabin3n, 64, 1, ald3),
                       next_coll=ag3)

            # ===== layer 3 (1 head, no concat, no relu)
            edge_phase(3, tabg[2], ROW3, 64, 1, xt3, b3r, False, ald3,
                       next_cb=classifier_blk)
    nc.compile()
    return nc


def kernel(x, edge_index, w1, as1, ad1, b1, w2, as2, ad2, b2,
           w3, as3, ad3, b3, wc, bc):
    x = np.asarray(x)
    ei = np.asarray(edge_index).astype(np.int64)
    loop = np.arange(N, dtype=np.int64)
    src = np.concatenate([ei[0], loop])
    dst = np.concatenate([ei[1], loop])

    Qb, per_core_ix, row_of_node, node_of_row, colmask = _prep_edges(src, dst)

    w1e = _fold_w(np.asarray(w1), np.asarray(as1), np.asarray(ad1)).astype(BF16)
    w2e = _fold_w(np.asarray(w2), np.asarray(as2), np.asarray(ad2)).astype(BF16)
    w3e = _fold_w(np.asarray(w3), np.asarray(as3), np.asarray(ad3)).astype(BF16)
    wc_b = np.asarray(wc).astype(BF16)
    b1r = np.tile(np.asarray(b1).astype(BF16)[None, :], (128, 1))
    b2r = np.tile(np.asarray(b2).astype(BF16)[None, :], (128, 1))
    b3r = np.tile(np.asarray(b3).astype(BF16)[None, :], (128, 1))
    bcr = np.tile(np.asarray(bc).astype(np.float32)[None, :], (128, 1))

    key = ("v5", tuple(Qb))
    if key not in _cache:
        _cache[key] = _build(Qb)
    nc = _cache[key]

    xbf = x.astype(BF16)
    in_maps = []
    for c in range(NCORE):
        xt = np.zeros((FIN, NPAD), BF16)
        rows = node_of_row[c]
        real = rows >= 0
        xt[:, real] = xbf[rows[real]].T
        in_maps.append({
            "xt": xt, "w1e": w1e, "w2e": w2e, "w3e": w3e, "wc": wc_b,
            "b1r": b1r, "b2r": b2r, "b3r": b3r, "bcr": bcr,
            "ixs": per_core_ix[c], "cm": colmask[c],
        })
    res = run_bass_kernel_spmd(nc, in_maps, core_ids=list(range(NCORE)))
    globals()["LAST_RESULTS"] = res
    out = np.empty((N, 4), np.float32)
    for c in range(NCORE):
        rows = node_of_row[c]
        real = rows >= 0
        out[rows[real]] = res.results[c]["out"][real]
    return out
